# revision 1
# baseline (speedup 1.0000x reference)
"""DeepseekV3 MLA decode attention kernel for 8 Trainium2 NeuronCores.

Sharding: 4 head-groups (32 heads each) x 2 batch-groups (8 batches each).
Each core computes the full attention output for its (head-group, batch-group)
tile. Weights are sharded by head, KV cache by batch. All matmul operands are
bf16 (fp32 PSUM accumulation); softmax runs in fp32.

Per-core pipeline:
  1. q = q_dn @ wq^T                (bf16 matmul, fp32 psum)
  2. RoPE on q_pe and new-token k_pe (DVE, fp32)
  3. PE transposes of q_nope/q_pe per head -> [d, b] layout
  4. absorption: q_lat^T[c,b] = w_ukv[h]^T-slices @ q_nope^T
  5. per batch: scores = q_lat.ckv^T + q_pe.kpe^T  -> exp (ACT, accum sums)
     -> transpose p -> o = p^T.T @ ckv -> scale by 1/sum
  6. out[b,d] = o^T-slices @ w_uv^T per head
"""

import sys

for _p in ("/opt/trn_rl_repo", "/root/.axon_site/_ro/trn_rl_repo"):
    if _p not in sys.path:
        sys.path.append(_p)

import numpy as np
import ml_dtypes

import concourse.bass as bass
import concourse.bacc as bacc
import concourse.tile as tile
from concourse import mybir
from concourse.bass_utils import run_bass_kernel_spmd
from concourse.masks import make_identity

BF16 = mybir.dt.bfloat16
FP8 = mybir.dt.float8e4
F32 = mybir.dt.float32
NPBF = ml_dtypes.bfloat16
NPF8 = ml_dtypes.float8_e4m3
FP8S = 16.0  # scale applied to fp8-stored tensors (q side and k side)

NUM_HEADS = 128
QK_NOPE = 128
QK_ROPE = 64
V_HEAD = 128
QD = 192  # q head dim (nope + rope)
C = 512   # kv lora rank
L = 1536  # q lora rank
SCALE = 1.0 / float(np.sqrt(192.0))

HG = 4  # head groups
BGQ = 2  # batch groups
N_CORES = 8

_BUILD_CACHE = {}


def _build(n_cached, B, H):
    """Build the per-core Bass program. Identical on all cores (pure SPMD)."""
    NT_T = n_cached // 128   # full 128-row n tiles (16)
    NCH = n_cached // 512    # 512-wide score chunks (4)
    HD = H * QD              # 6144
    LT = L // 128            # 12
    NJ = HD // 512           # 12
    assert n_cached % 512 == 0

    nc = bacc.Bacc("TRN2", target_bir_lowering=False, debug=False)

    q_dnT = nc.dram_tensor("q_dnT", [L, B], FP8, kind="ExternalInput")
    wqT = nc.dram_tensor("wqT", [L, HD], FP8, kind="ExternalInput")
    w_ukv = nc.dram_tensor("w_ukv", [H, QK_NOPE, C], FP8, kind="ExternalInput")
    w_uvT = nc.dram_tensor("w_uvT", [H, 128, 4, V_HEAD], BF16, kind="ExternalInput")
    ckv = nc.dram_tensor("ckv", [B, n_cached, C], BF16, kind="ExternalInput")
    ckvT = nc.dram_tensor("ckvT", [B, C, n_cached], FP8, kind="ExternalInput")
    kpeT = nc.dram_tensor("kpeT", [B, QK_ROPE, n_cached], FP8, kind="ExternalInput")
    ckv_new = nc.dram_tensor("ckv_new", [1, B, C], BF16, kind="ExternalInput")
    ckv_newT = nc.dram_tensor("ckv_newT", [C, B], FP8, kind="ExternalInput")
    kpe_new = nc.dram_tensor("kpe_new", [B, QK_ROPE], F32, kind="ExternalInput")
    cos_rep = nc.dram_tensor("cos_rep", [B, H * 32], F32, kind="ExternalInput")
    sin_rep = nc.dram_tensor("sin_rep", [B, H * 32], F32, kind="ExternalInput")
    out = nc.dram_tensor("out", [B, H, V_HEAD], F32, kind="ExternalOutput")

    with tile.TileContext(nc) as tc:
        # Outer (whole-kernel-lifetime) pools. The big cache-streaming pools
        # are opened first so their SBUF addresses never overlap the phase-A
        # scratch pools -> their DMAs can start at t=0.
        with (
            tc.tile_pool(name="ckvT_p", bufs=4) as ckvT_p,
            tc.tile_pool(name="ckv_p", bufs=4) as ckv_p,
            tc.tile_pool(name="kpeT_p", bufs=2) as kpeT_p,
            tc.tile_pool(name="consts", bufs=1) as consts,
            tc.tile_pool(name="persist", bufs=1) as persist,
        ):
            ident = consts.tile([128, 128], BF16)
            make_identity(nc, ident)
            cos_sb = consts.tile([B, H * 32], F32)
            nc.sync.dma_start(out=cos_sb, in_=cos_rep[:, :])
            sin_sb = consts.tile([B, H * 32], F32)
            nc.sync.dma_start(out=sin_sb, in_=sin_rep[:, :])
            kpnew_sb = consts.tile([B, QK_ROPE], F32)
            nc.sync.dma_start(out=kpnew_sb, in_=kpe_new[:, :])
            qdn_sb = consts.tile([128, LT, B], FP8)
            nc.sync.dma_start(
                out=qdn_sb, in_=q_dnT[:, :].rearrange("(t p) b -> p t b", p=128)
            )
            ckvnewT_sb = consts.tile([128, 4, B], FP8)
            nc.sync.dma_start(
                out=ckvnewT_sb, in_=ckv_newT[:, :].rearrange("(ct p) b -> p ct b", p=128)
            )
            ckvnew_sb2 = consts.tile([1, B, C], BF16)
            nc.sync.dma_start(out=ckvnew_sb2, in_=ckv_new[:, :, :])

            # persistent intermediates
            qlatT = persist.tile([128, H, 4, B], FP8)
            qpeT = persist.tile([QK_ROPE, H, B], FP8)
            knewT = persist.tile([QK_ROPE, B], FP8)

            # ---------------- Phase A: q projection, rope, transposes -------
            with (
                tc.tile_pool(name="s1a", bufs=1) as s1a,
                tc.tile_pool(name="wq_p", bufs=16) as wq_p,
                tc.tile_pool(name="wukv_p", bufs=4) as wukv_p,
                tc.tile_pool(name="ps_q", bufs=2, space="PSUM") as ps_q,
                tc.tile_pool(name="ps_t", bufs=2, space="PSUM") as ps_t,
            ):
                q_sb = s1a.tile([B, HD], BF16)
                JG = 4  # j's per wq column group
                for jg in range(NJ // JG):
                    wq_tiles = []
                    for t in range(LT):
                        wqt = wq_p.tile([128, JG * 512], FP8, tag="wq", name=f"wqt{jg}_{t}")
                        nc.sync.dma_start(
                            out=wqt,
                            in_=wqT[t * 128:(t + 1) * 128,
                                    jg * JG * 512:(jg + 1) * JG * 512],
                        )
                        wq_tiles.append(wqt)
                    for jj in range(JG):
                        j = jg * JG + jj
                        psq = ps_q.tile([B, 512], F32, tag="psq")
                        for t in range(LT):
                            nc.tensor.matmul(
                                psq, lhsT=qdn_sb[:, t, :],
                                rhs=wq_tiles[t][:, jj * 512:(jj + 1) * 512],
                                start=(t == 0), stop=(t == LT - 1),
                            )
                        nc.vector.tensor_copy(q_sb[:, j * 512:(j + 1) * 512], psq)

                qv = q_sb.rearrange("b (h d) -> b h d", d=QD)
                # rope on q_pe: interleaved pairs -> half-split rotated layout
                xpairs = qv[:, :, QK_NOPE:].rearrange("b h (i two) -> b h i two", two=2)
                xe = xpairs[:, :, :, 0]
                xo = xpairs[:, :, :, 1]
                cos3 = cos_sb.rearrange("b (h i) -> b h i", i=32)
                sin3 = sin_sb.rearrange("b (h i) -> b h i", i=32)
                qpe_bf = s1a.tile([B, H, QK_ROPE], BF16)
                tmp = s1a.tile([B, 4, H, 32], F32)
                nc.vector.tensor_mul(tmp[:, 0], xe, cos3)
                nc.vector.tensor_mul(tmp[:, 1], xo, sin3)
                nc.vector.tensor_sub(qpe_bf[:, :, 0:32], tmp[:, 0], tmp[:, 1])
                nc.vector.tensor_mul(tmp[:, 2], xo, cos3)
                nc.vector.tensor_mul(tmp[:, 3], xe, sin3)
                nc.vector.tensor_add(qpe_bf[:, :, 32:64], tmp[:, 2], tmp[:, 3])

                # rope on the new-token k_pe
                kpairs = kpnew_sb.rearrange("b (i two) -> b i two", two=2)
                kxe = kpairs[:, :, 0]
                kxo = kpairs[:, :, 1]
                kr_bf = s1a.tile([B, QK_ROPE], BF16)
                ktmp = s1a.tile([B, 4, 32], F32)
                nc.vector.tensor_mul(ktmp[:, 0], kxe, cos3[:, 0, :])
                nc.vector.tensor_mul(ktmp[:, 1], kxo, sin3[:, 0, :])
                nc.vector.tensor_sub(kr_bf[:, 0:32], ktmp[:, 0], ktmp[:, 1])
                nc.vector.tensor_mul(ktmp[:, 2], kxo, cos3[:, 0, :])
                nc.vector.tensor_mul(ktmp[:, 3], kxe, sin3[:, 0, :])
                nc.vector.tensor_add(kr_bf[:, 32:64], ktmp[:, 2], ktmp[:, 3])

                # transposes: [B, d] -> [d, B], grouped 8 heads per psum tile
                TCH = 8
                qnT = s1a.tile([128, H, B], FP8)
                for hc in range(H // TCH):
                    ptn = ps_t.tile([128, TCH, B], BF16, tag="tr")
                    ptp2 = ps_t.tile([128, TCH, B], BF16, tag="tr")
                    for hh in range(TCH):
                        h = hc * TCH + hh
                        nc.tensor.transpose(ptn[:, hh, :], qv[:, h, 0:QK_NOPE], ident[:B, :B])
                        nc.tensor.transpose(ptp2[:QK_ROPE, hh, :], qpe_bf[:, h, :], ident[:B, :B])
                    nc.vector.tensor_copy(qnT[:, hc * TCH:(hc + 1) * TCH, :], ptn)
                    nc.vector.tensor_copy(qpeT[:, hc * TCH:(hc + 1) * TCH, :], ptp2[:QK_ROPE])
                ptk = ps_t.tile([128, TCH, B], BF16, tag="tr")
                nc.tensor.transpose(ptk[:QK_ROPE, 0, :], kr_bf, ident[:B, :B])
                nc.vector.tensor_copy(knewT, ptk[:QK_ROPE, 0, :])

                # absorption: q_latT[c, b]; 8 heads x 4 ct per psum tile
                HCH = 8
                for hc in range(H // HCH):
                    wut = wukv_p.tile([128, HCH, C], FP8, tag="wukv")
                    nc.sync.dma_start(
                        out=wut,
                        in_=w_ukv[hc * HCH:(hc + 1) * HCH].rearrange("h d c -> d h c"),
                    )
                    pa = ps_t.tile([128, HCH, 4, B], F32, tag="abs")
                    for hh in range(HCH):
                        h = hc * HCH + hh
                        for ct in range(4):
                            nc.tensor.matmul(
                                pa[:, hh, ct, :],
                                lhsT=wut[:, hh, ct * 128:(ct + 1) * 128],
                                rhs=qnT[:, h, :], start=True, stop=True,
                            )
                    nc.vector.tensor_scalar_mul(
                        qlatT[:, hc * HCH:(hc + 1) * HCH, :, :], pa, 1.0 / FP8S
                    )

            # ---------------- Phase B: attention per batch ------------------
            with (
                tc.tile_pool(name="p_p", bufs=2) as p_p,
                tc.tile_pool(name="pT_p", bufs=2) as pT_p,
                tc.tile_pool(name="o_p", bufs=2) as o_p,
                tc.tile_pool(name="oT_p", bufs=1) as oT_p,
                tc.tile_pool(name="sum_p", bufs=2) as sum_p,
                tc.tile_pool(name="wuv_p", bufs=4) as wuv_p,
                tc.tile_pool(name="outs_p", bufs=1) as outs_p,
                tc.tile_pool(name="ps_s", bufs=2, space="PSUM") as ps_s,
                tc.tile_pool(name="ps_pt", bufs=2, space="PSUM") as ps_pt,
                tc.tile_pool(name="ps_o", bufs=1, space="PSUM") as ps_o,
                tc.tile_pool(name="ps_r", bufs=2, space="PSUM") as ps_r,
            ):
                oT = oT_p.tile([128, 4, H, B], BF16)
                kpe_tiles = {}
                for b in range(B):
                    ckvT_t = ckvT_p.tile([128, 4, n_cached], FP8, tag="ckvT")
                    nc.sync.dma_start(
                        out=ckvT_t,
                        in_=ckvT[b].rearrange("(ct p) n -> p ct n", p=128),
                    )
                    if b % 2 == 0:
                        kpeT_t2 = kpeT_p.tile([QK_ROPE, 2, n_cached], FP8, tag="kpeT")
                        nc.sync.dma_start(
                            out=kpeT_t2,
                            in_=kpeT[b:b + 2].rearrange("b j n -> j b n"),
                        )
                    kpeT_t = kpeT_t2[:, b % 2, :]

                    p_bf = p_p.tile([32, n_cached], BF16, tag="p")
                    p_tail = p_p.tile([32, 1], BF16, tag="ptail")
                    sums = sum_p.tile([32, 8], F32, tag="sums")
                    # scores + exp, 512-wide chunks
                    for nch in range(NCH):
                        pss = ps_s.tile([32, 512], F32, tag="s")
                        for ct in range(4):
                            nc.tensor.matmul(
                                pss,
                                lhsT=qlatT[:, :, ct, b],
                                rhs=ckvT_t[:, ct, nch * 512:(nch + 1) * 512],
                                start=(ct == 0), stop=False,
                            )
                        nc.tensor.matmul(
                            pss, lhsT=qpeT[:, :, b],
                            rhs=kpeT_t[:, nch * 512:(nch + 1) * 512],
                            start=False, stop=True,
                        )
                        nc.scalar.activation(
                            p_bf[:, nch * 512:(nch + 1) * 512], pss,
                            mybir.ActivationFunctionType.Exp,
                            scale=SCALE / (FP8S * FP8S * FP8S), accum_out=sums[:, nch:nch + 1],
                        )
                    # new-token column
                    pst = ps_s.tile([32, 512], F32, tag="s")
                    for ct in range(4):
                        nc.tensor.matmul(
                            pst[:, 0:1], lhsT=qlatT[:, :, ct, b],
                            rhs=ckvnewT_sb[:, ct, b:b + 1],
                            start=(ct == 0), stop=False,
                        )
                    nc.tensor.matmul(
                        pst[:, 0:1], lhsT=qpeT[:, :, b], rhs=knewT[:, b:b + 1],
                        start=False, stop=True,
                    )
                    nc.scalar.activation(
                        p_tail, pst[:, 0:1],
                        mybir.ActivationFunctionType.Exp,
                        scale=SCALE / (FP8S * FP8S * FP8S), accum_out=sums[:, NCH:NCH + 1],
                    )
                    # 1 / sum
                    ssum = sum_p.tile([32, 1], F32, tag="ssum")
                    nc.vector.reduce_sum(ssum, sums[:, 0:NCH + 1], axis=mybir.AxisListType.X)
                    rcp = sum_p.tile([32, 1], F32, tag="rcp")
                    nc.vector.reciprocal(rcp, ssum)

                    # transpose p -> pT tiles (4 per psum tile)
                    pT = pT_p.tile([128, NT_T, 32], BF16, tag="pT")
                    for g in range(NT_T // 4):
                        ptp = ps_pt.tile([128, 4, 32], BF16, tag="pt")
                        for k in range(4):
                            nt = g * 4 + k
                            nc.tensor.transpose(
                                ptp[:, k, :], p_bf[:, nt * 128:(nt + 1) * 128],
                                ident[:32, :32],
                            )
                        nc.vector.tensor_copy(pT[:, g * 4:(g + 1) * 4, :], ptp)
                    ptt = ps_pt.tile([128, 4, 32], BF16, tag="pt")
                    nc.tensor.transpose(ptt[0:1, 0, :], p_tail, ident[:32, :32])
                    pT_tail = pT_p.tile([1, 32], BF16, tag="pTt")
                    nc.vector.tensor_copy(pT_tail, ptt[0:1, 0, :])

                    # o = p @ ckv   (accumulate over n tiles)
                    pso = ps_o.tile([32, C], F32, tag="o")
                    for g in range(NT_T // 4):
                        ckv_t = ckv_p.tile([128, 4, C], BF16, tag="ckv")
                        nc.sync.dma_start(
                            out=ckv_t,
                            in_=ckv[b, g * 512:(g + 1) * 512, :].rearrange(
                                "(nt p) c -> p nt c", p=128
                            ),
                        )
                        for k in range(4):
                            nt = g * 4 + k
                            nc.tensor.matmul(
                                pso, lhsT=pT[:, nt, :], rhs=ckv_t[:, k, :],
                                start=(nt == 0), stop=False,
                            )
                    nc.tensor.matmul(
                        pso, lhsT=pT_tail, rhs=ckvnew_sb2[:, b, :],
                        start=False, stop=True,
                    )
                    # o / sum -> bf16
                    o_bf = o_p.tile([32, C], BF16, tag="obf")
                    nc.vector.tensor_scalar_mul(o_bf, pso, rcp)
                    # transpose o -> oT[:, :, :, b]
                    pto = ps_pt.tile([128, 4, 32], BF16, tag="pt")
                    for ct in range(4):
                        nc.tensor.transpose(
                            pto[:, ct, :], o_bf[:, ct * 128:(ct + 1) * 128],
                            ident[:32, :32],
                        )
                    nc.vector.tensor_copy(oT[:, :, :, b], pto)

                # ---------------- output projection ------------------------
                out_sb = outs_p.tile([B, H, V_HEAD], F32)
                HCH = 8
                for hc in range(H // HCH):
                    wvt = wuv_p.tile([128, HCH, 4, V_HEAD], BF16, tag="wuv")
                    nc.sync.dma_start(
                        out=wvt,
                        in_=w_uvT[hc * HCH:(hc + 1) * HCH].rearrange("h p ct d -> p h ct d"),
                    )
                    for hh in range(HCH):
                        h = hc * HCH + hh
                        psr = ps_r.tile([B, V_HEAD], F32, tag="r")
                        for ct in range(4):
                            nc.tensor.matmul(
                                psr, lhsT=oT[:, ct, h, :], rhs=wvt[:, hh, ct, :],
                                start=(ct == 0), stop=(ct == 3),
                            )
                        nc.vector.tensor_copy(out_sb[:, h, :], psr)
                    nc.sync.dma_start(
                        out=out[:, hc * HCH:(hc + 1) * HCH, :],
                        in_=out_sb[:, hc * HCH:(hc + 1) * HCH, :],
                    )

    nc.compile()
    return nc


def _get_build(n_cached, B, H):
    key = (n_cached, B, H)
    if key not in _BUILD_CACHE:
        _BUILD_CACHE[key] = _build(n_cached, B, H)
    return _BUILD_CACHE[key]


def prepare_in_maps(**inputs):
    """Host-side sharding / layout prep. Returns (in_maps, meta)."""
    q = np.asarray(inputs["q_normed_dn"], dtype=np.float32)      # [16,1,1536]
    ckv_new = np.asarray(inputs["compressed_kv"], dtype=np.float32)  # [16,1,512]
    k_pe = np.asarray(inputs["k_pe"], dtype=np.float32)          # [16,1,1,64]
    pos = np.asarray(inputs["position_ids"]).astype(np.int64)    # [16,1]
    start_pos = int(inputs["start_pos"])
    ckv_cache = np.asarray(inputs["ckv_cache"], dtype=np.float32)
    kpe_cache = np.asarray(inputs["k_pe_cache"], dtype=np.float32)
    sin_c = np.asarray(inputs["sin_cache"], dtype=np.float32)
    cos_c = np.asarray(inputs["cos_cache"], dtype=np.float32)
    wkv_b = np.asarray(inputs["wkv_b"], dtype=np.float32)        # [128,256,512]
    wq_b = np.asarray(inputs["wq_b"], dtype=np.float32)          # [24576,1536]

    bsz = q.shape[0]
    B = bsz // BGQ
    H = NUM_HEADS // HG
    n_cached = start_pos

    cos_g = cos_c[pos[:, 0]][:, :32]                             # [16,32]
    sin_g = sin_c[pos[:, 0]][:, :32]
    cos_rep = np.tile(cos_g, (1, H)).astype(np.float32)          # [16,H*32]
    sin_rep = np.tile(sin_g, (1, H)).astype(np.float32)

    wq_r = wq_b.reshape(NUM_HEADS, QD, L)

    # per head-group weights
    wq_shards, wukv_shards, wuv_shards = [], [], []
    for hg in range(HG):
        hs = slice(hg * H, (hg + 1) * H)
        wq_shards.append(
            np.ascontiguousarray(wq_r[hs].reshape(H * QD, L).T * FP8S).astype(NPF8)
        )
        wukv_shards.append(np.ascontiguousarray(wkv_b[hs, :QK_NOPE, :] * FP8S).astype(NPF8))
        # w_uv pre-arranged to the SBUF tile layout [H, p, ct, d] so every
        # DMA descriptor run is >= 1KB contiguous
        wuvT = wkv_b[hs, QK_NOPE:, :].transpose(0, 2, 1)          # [H, C, D]
        wuv_shards.append(
            np.ascontiguousarray(
                wuvT.reshape(H, 4, 128, V_HEAD).transpose(0, 2, 1, 3)
            ).astype(NPBF)
        )

    # per batch-group caches
    ckv_shards, ckvT_shards, kpeT_shards = [], [], []
    qT_shards, ckvnew_shards, ckvnewT_shards, kpnew_shards = [], [], [], []
    cos_shards, sin_shards = [], []
    for bg in range(BGQ):
        bs = slice(bg * B, (bg + 1) * B)
        ckv_shards.append(np.ascontiguousarray(ckv_cache[bs, :n_cached, :]).astype(NPBF))
        ckvT_shards.append(
            np.ascontiguousarray(ckv_cache[bs, :n_cached, :].transpose(0, 2, 1) * FP8S).astype(NPF8)
        )
        kpeT_shards.append(
            np.ascontiguousarray(kpe_cache[bs, :n_cached, :].transpose(0, 2, 1) * FP8S).astype(NPF8)
        )
        qT_shards.append(np.ascontiguousarray(q[bs, 0, :].T * FP8S).astype(NPF8))
        ckvnew_shards.append(ckv_new[bs, 0, :].astype(NPBF).reshape(1, B, C))
        ckvnewT_shards.append(np.ascontiguousarray(ckv_new[bs, 0, :].T * FP8S).astype(NPF8))
        kpnew_shards.append(np.ascontiguousarray(k_pe[bs, 0, 0, :] * FP8S).astype(np.float32))
        cos_shards.append(np.ascontiguousarray(cos_rep[bs]))
        sin_shards.append(np.ascontiguousarray(sin_rep[bs]))

    in_maps = []
    for core in range(N_CORES):
        hg, bg = core // BGQ, core % BGQ
        in_maps.append({
            "q_dnT": qT_shards[bg],
            "wqT": wq_shards[hg],
            "w_ukv": wukv_shards[hg],
            "w_uvT": wuv_shards[hg],
            "ckv": ckv_shards[bg],
            "ckvT": ckvT_shards[bg],
            "kpeT": kpeT_shards[bg],
            "ckv_new": ckvnew_shards[bg],
            "ckv_newT": ckvnewT_shards[bg],
            "kpe_new": kpnew_shards[bg],
            "cos_rep": cos_shards[bg],
            "sin_rep": sin_shards[bg],
        })
    return in_maps, (n_cached, B, H, bsz)


def assemble(results, meta):
    n_cached, B, H, bsz = meta
    out_full = np.empty((bsz, NUM_HEADS, V_HEAD), dtype=np.float32)
    for core in range(N_CORES):
        hg, bg = core // BGQ, core % BGQ
        out_full[bg * B:(bg + 1) * B, hg * H:(hg + 1) * H, :] = results[core]["out"]
    return out_full


def kernel(**inputs):
    in_maps, meta = prepare_in_maps(**inputs)
    n_cached, B, H, bsz = meta
    nc = _get_build(n_cached, B, H)
    res = run_bass_kernel_spmd(nc, in_maps, core_ids=list(range(N_CORES)))
    return assemble(res.results, meta)



# revision 2
# speedup vs baseline: 8.8121x; 8.8121x over previous
"""DeepseekV3 MLA decode attention kernel for 8 Trainium2 NeuronCores, v2.

Two-phase sharding with on-device AllToAll:
  Phase 1 (head-sharded, 16 heads/core): q projection (fp8 DoubleRow),
    rope, absorption -> q_full[b, h, 640] fp8 (lat 512 | rope 64 | pad).
  AllToAll #1: redistribute q to batch-sharding (2 batches/core).
  Phase 2 (batch-sharded): scores for all 128 heads (M=128 matmuls,
    fp8 DoubleRow vs fused ckvT/kpeT cache), exp, o = p @ ckv (bf16).
  AllToAll #2: redistribute o back to head-sharding.
  Phase 3 (head-sharded): output projection o @ w_uv per head.
"""

import sys

for _p in ("/opt/trn_rl_repo", "/root/.axon_site/_ro/trn_rl_repo"):
    if _p not in sys.path:
        sys.path.append(_p)

import numpy as np
import ml_dtypes

import concourse.bass as bass
import concourse.bacc as bacc
import concourse.tile as tile
from concourse import mybir
from concourse.bass_utils import run_bass_kernel_spmd
from concourse.masks import make_identity

BF16 = mybir.dt.bfloat16
FP8 = mybir.dt.float8e4
F32 = mybir.dt.float32
NPBF = ml_dtypes.bfloat16
NPF8 = ml_dtypes.float8_e4m3
DR = mybir.MatmulPerfMode.DoubleRow
FP8S = 16.0  # scale applied to fp8-stored tensors

NUM_HEADS = 128
QK_NOPE = 128
QK_ROPE = 64
V_HEAD = 128
QD = 192   # q head dim (nope + rope)
C = 512    # kv lora rank
CF = 640   # fused row count: 512 lat + 64 rope + 64 pad
L = 1536   # q lora rank
SCALE = 1.0 / float(np.sqrt(192.0))

N_CORES = 8
HL = NUM_HEADS // N_CORES   # 16 local heads
BL = 2                      # 2 local batches
B_ALL = 16

_BUILD_CACHE = {}


def _build(n_cached, sim=False):
    """Per-core Bass program (pure SPMD; core identity lives in the shards)."""
    NT = n_cached // 128     # 16 full n tiles
    NCH = n_cached // 512    # 4 score chunks
    LT = L // 128            # 12
    HD = HL * QD             # 3072
    NJ = HD // 512           # 6
    assert n_cached % 512 == 0

    nc = bacc.Bacc("TRN2", target_bir_lowering=False, debug=False,
                   num_devices=N_CORES)

    qdn = nc.dram_tensor("qdn", [128, LT * B_ALL], FP8, kind="ExternalInput")
    wqT = nc.dram_tensor("wqT", [L, HD], FP8, kind="ExternalInput")
    w_ukv = nc.dram_tensor("w_ukv", [HL, QK_NOPE, C], FP8, kind="ExternalInput")
    w_uvT = nc.dram_tensor("w_uvT", [128, HL, 4, V_HEAD], BF16, kind="ExternalInput")
    fkv = nc.dram_tensor("fkv", [BL, CF, n_cached], FP8, kind="ExternalInput")
    ckv16 = nc.dram_tensor("ckv16", [BL, n_cached, C], BF16, kind="ExternalInput")
    fnew = nc.dram_tensor("fnew", [128, 5 * BL], FP8, kind="ExternalInput")
    ckv_new16 = nc.dram_tensor("ckv_new16", [1, BL, C], BF16, kind="ExternalInput")
    cosq = nc.dram_tensor("cosq", [B_ALL, HL * 32], F32, kind="ExternalInput")
    sinq = nc.dram_tensor("sinq", [B_ALL, HL * 32], F32, kind="ExternalInput")
    out = nc.dram_tensor("out", [B_ALL, HL, V_HEAD], F32, kind="ExternalOutput")

    with tile.TileContext(nc) as tc:
        with (
            # cache-streaming pools first so their DMAs start at t=0
            tc.tile_pool(name="fkv_p", bufs=1) as fkv_p,
            tc.tile_pool(name="ckv16_p", bufs=1) as ckv16_p,
            tc.tile_pool(name="wuv_p", bufs=1) as wuv_p,
            tc.tile_pool(name="consts", bufs=1) as consts,
            tc.tile_pool(name="persist", bufs=1) as persist,
            tc.tile_pool(name="dram", bufs=1, space="DRAM") as dram,
        ):
            fkv_sb = fkv_p.tile([128, BL, 5, n_cached], FP8)
            for b in range(BL):
                nc.sync.dma_start(
                    out=fkv_sb[:, b], in_=fkv[b].rearrange("(ct p) n -> p ct n", p=128)
                )
            ckv16_sb = ckv16_p.tile([128, BL, NT, C], BF16)
            for b in range(BL):
                nc.sync.dma_start(
                    out=ckv16_sb[:, b],
                    in_=ckv16[b].rearrange("(nt p) c -> p nt c", p=128),
                )
            wuv_sb = wuv_p.tile([128, HL, 4, V_HEAD], BF16)
            nc.sync.dma_start(out=wuv_sb, in_=w_uvT[:, :, :, :])

            ident = consts.tile([128, 128], BF16)
            make_identity(nc, ident)
            qdn_sb = consts.tile([128, LT, B_ALL], FP8)
            nc.sync.dma_start(out=qdn_sb, in_=qdn.rearrange("p (t b) -> p t b", t=LT))
            fnew_sb = consts.tile([128, 5, BL], FP8)
            nc.sync.dma_start(out=fnew_sb, in_=fnew.rearrange("p (ct b) -> p ct b", ct=5))
            ckvnew_sb = consts.tile([1, BL, C], BF16)
            nc.sync.dma_start(out=ckvnew_sb, in_=ckv_new16[:, :, :])
            cos_sb = consts.tile([B_ALL, HL * 32], F32)
            nc.sync.dma_start(out=cos_sb, in_=cosq[:, :])
            sin_sb = consts.tile([B_ALL, HL * 32], F32)
            nc.sync.dma_start(out=sin_sb, in_=sinq[:, :])

            a2a_q_in = dram.tile([N_CORES, BL, HL, CF], BF16)
            a2a_q_out = dram.tile([N_CORES, BL, HL, CF], BF16)
            a2a_o_in = dram.tile([N_CORES, BL, HL, C], BF16)
            a2a_o_out = dram.tile([N_CORES, BL, HL, C], BF16)

            # ------------- Phase 1: qproj + rope + absorption (head-shard) ---
            with (
                tc.tile_pool(name="s1", bufs=1) as s1,
                tc.tile_pool(name="wq_p", bufs=3) as wq_p,
                tc.tile_pool(name="wukv_p", bufs=1) as wukv_p,
                tc.tile_pool(name="ps_q", bufs=2, space="PSUM") as ps_q,
                tc.tile_pool(name="ps_t", bufs=2, space="PSUM") as ps_t,
                tc.tile_pool(name="ps_a", bufs=2, space="PSUM") as ps_a,
            ):
                wukv_sb = wukv_p.tile([128, HL, C], FP8)
                nc.sync.dma_start(out=wukv_sb, in_=w_ukv.rearrange("h d c -> d h c"))

                q_sb = s1.tile([B_ALL, HL, QD], BF16)
                qv = q_sb.rearrange("b h d -> b (h d)")
                for j in range(NJ):
                    wqt = wq_p.tile([128, LT, 512], FP8, tag="wq", name=f"wq{j}")
                    nc.sync.dma_start(
                        out=wqt,
                        in_=wqT[:, j * 512:(j + 1) * 512].rearrange(
                            "(t p) n -> p t n", p=128
                        ),
                    )
                    psq = ps_q.tile([B_ALL, 512], F32, tag="psq")
                    for tp in range(LT // 2):
                        nc.tensor.matmul(
                            psq,
                            lhsT=qdn_sb[:, 2 * tp:2 * tp + 2, :],
                            rhs=wqt[:, 2 * tp:2 * tp + 2, :],
                            start=(tp == 0), stop=(tp == LT // 2 - 1),
                            perf_mode=DR,
                        )
                    nc.vector.tensor_scalar_mul(
                        qv[:, j * 512:(j + 1) * 512], psq, 1.0 / (FP8S * FP8S)
                    )

                # q_full: [16 b, 16 h, 640] fp8 at x16 scale
                qfull = s1.tile([B_ALL, HL, CF], BF16)
                nc.vector.memset(qfull[:, :, C + QK_ROPE:], 0.0)

                # rope on q_pe (interleaved pairs -> half-split rotated),
                # cos/sin are pre-scaled by FP8S on host
                xpairs = q_sb[:, :, QK_NOPE:].rearrange("b h (i two) -> b h i two", two=2)
                xe = xpairs[:, :, :, 0]
                xo = xpairs[:, :, :, 1]
                cos3 = cos_sb.rearrange("b (h i) -> b h i", i=32)
                sin3 = sin_sb.rearrange("b (h i) -> b h i", i=32)
                qpe_v = qfull[:, :, C:C + QK_ROPE]
                tmp = s1.tile([B_ALL, 4, HL, 32], F32)
                nc.vector.tensor_mul(tmp[:, 0], xe, cos3)
                nc.vector.tensor_mul(tmp[:, 1], xo, sin3)
                nc.vector.tensor_sub(qpe_v[:, :, 0:32], tmp[:, 0], tmp[:, 1])
                nc.vector.tensor_mul(tmp[:, 2], xo, cos3)
                nc.vector.tensor_mul(tmp[:, 3], xe, sin3)
                nc.vector.tensor_add(qpe_v[:, :, 32:64], tmp[:, 2], tmp[:, 3])

                # absorption: per head transpose q_nope then q_lat = qnT^T @ w_ukv
                qnT = s1.tile([128, HL, B_ALL], BF16)
                for hc in range(HL // 8):
                    ptn = ps_t.tile([128, 8, B_ALL], BF16, tag="tr")
                    for hh in range(8):
                        h = hc * 8 + hh
                        nc.tensor.transpose(
                            ptn[:, hh, :], q_sb[:, h, 0:QK_NOPE],
                            ident[:B_ALL, :B_ALL],
                        )
                    nc.vector.tensor_copy(qnT[:, hc * 8:(hc + 1) * 8, :], ptn)
                for h in range(HL):
                    pa = ps_a.tile([B_ALL, C], F32, tag="abs")
                    nc.tensor.matmul(
                        pa, lhsT=qnT[:, h, :], rhs=wukv_sb[:, h, :],
                        start=True, stop=True,
                    )
                    nc.vector.tensor_copy(qfull[:, h, 0:C], pa)

                # ship q to batch-sharding
                nc.sync.dma_start(
                    out=a2a_q_in.rearrange("d b h c -> (d b) h c"), in_=qfull
                )

            if sim:
                nc.gpsimd.dma_start(a2a_q_out[:], a2a_q_in[:])
            else:
                nc.gpsimd.collective_compute(
                    "AllToAll", mybir.AluOpType.bypass,
                    replica_groups=[list(range(N_CORES))],
                    ins=[a2a_q_in.opt()], outs=[a2a_q_out.opt()],
                )

            # ------------- Phase 2: attention (batch-shard, all 128 heads) ---
            with (
                tc.tile_pool(name="s2", bufs=1) as s2,
                tc.tile_pool(name="p_p", bufs=2) as p_p,
                tc.tile_pool(name="pT_p", bufs=2) as pT_p,
                tc.tile_pool(name="sum_p", bufs=2) as sum_p,
                tc.tile_pool(name="ps_s", bufs=2, space="PSUM") as ps_s,
                tc.tile_pool(name="ps_pt", bufs=2, space="PSUM") as ps_pt,
                tc.tile_pool(name="ps_o", bufs=2, space="PSUM") as ps_o,
            ):
                qh_sb = s2.tile([128, BL, CF], BF16)
                for s in range(N_CORES):
                    nc.sync.dma_start(
                        out=qh_sb[s * HL:(s + 1) * HL],
                        in_=a2a_q_out[s].rearrange("b h c -> h b c"),
                    )
                # transpose q to [c, heads] per batch
                qT = s2.tile([128, 5, BL, 128], FP8)
                for b in range(BL):
                    ptq = ps_pt.tile([128, 5, 128], BF16, tag="ptq")
                    for ct in range(5):
                        nc.tensor.transpose(
                            ptq[:, ct, :], qh_sb[:, b, ct * 128:(ct + 1) * 128],
                            ident,
                        )
                    nc.vector.tensor_copy(qT[:, :, b, :], ptq)

                o_sb = s2.tile([128, BL, C], BF16)
                for b in range(BL):
                    p_bf = p_p.tile([128, n_cached], BF16, tag="p")
                    p_tail = p_p.tile([128, 1], BF16, tag="ptail")
                    sums = sum_p.tile([128, NCH + 1], F32, tag="sums")
                    for nch in range(NCH):
                        pss = ps_s.tile([128, 512], F32, tag="s")
                        ns = slice(nch * 512, (nch + 1) * 512)
                        for cp in range(2):
                            nc.tensor.matmul(
                                pss,
                                lhsT=qT[:, 2 * cp:2 * cp + 2, b, :],
                                rhs=fkv_sb[:, b, 2 * cp:2 * cp + 2, ns],
                                start=(cp == 0), stop=False, perf_mode=DR,
                            )
                        nc.tensor.matmul(
                            pss, lhsT=qT[0:QK_ROPE, 4, b, :],
                            rhs=fkv_sb[0:QK_ROPE, b, 4, ns],
                            start=False, stop=True,
                        )
                        nc.scalar.activation(
                            p_bf[:, ns], pss,
                            mybir.ActivationFunctionType.Exp,
                            scale=SCALE / (FP8S * FP8S),
                            accum_out=sums[:, nch:nch + 1],
                        )
                    # new-token column
                    pst = ps_s.tile([128, 512], F32, tag="s")
                    for ct in range(4):
                        nc.tensor.matmul(
                            pst[:, 0:1], lhsT=qT[:, ct, b, :],
                            rhs=fnew_sb[:, ct, b:b + 1],
                            start=(ct == 0), stop=False,
                        )
                    nc.tensor.matmul(
                        pst[:, 0:1], lhsT=qT[0:QK_ROPE, 4, b, :],
                        rhs=fnew_sb[0:QK_ROPE, 4, b:b + 1],
                        start=False, stop=True,
                    )
                    nc.scalar.activation(
                        p_tail, pst[:, 0:1],
                        mybir.ActivationFunctionType.Exp,
                        scale=SCALE / (FP8S * FP8S),
                        accum_out=sums[:, NCH:NCH + 1],
                    )
                    ssum = sum_p.tile([128, 1], F32, tag="ssum")
                    nc.vector.reduce_sum(ssum, sums, axis=mybir.AxisListType.X)
                    rcp = sum_p.tile([128, 1], F32, tag="rcp")
                    nc.vector.reciprocal(rcp, ssum)

                    # transpose p -> pT
                    pT = pT_p.tile([128, NT, 128], BF16, tag="pT")
                    for g in range(NT // 4):
                        ptp = ps_pt.tile([128, 4, 128], BF16, tag="pt")
                        for k in range(4):
                            nt = g * 4 + k
                            nc.tensor.transpose(
                                ptp[:, k, :], p_bf[:, nt * 128:(nt + 1) * 128],
                                ident,
                            )
                        nc.vector.tensor_copy(pT[:, g * 4:(g + 1) * 4, :], ptp)
                    ptt = ps_pt.tile([128, 4, 128], BF16, tag="pt")
                    nc.tensor.transpose(ptt[0:1, 0, :], p_tail, ident)
                    pT_tail = pT_p.tile([1, 128], BF16, tag="pTt")
                    nc.vector.tensor_copy(pT_tail, ptt[0:1, 0, :])

                    # o = p @ ckv (+ new-token term), then / sum
                    pso = ps_o.tile([128, C], F32, tag="o")
                    for nt in range(NT):
                        nc.tensor.matmul(
                            pso, lhsT=pT[:, nt, :], rhs=ckv16_sb[:, b, nt, :],
                            start=(nt == 0), stop=False,
                        )
                    nc.tensor.matmul(
                        pso, lhsT=pT_tail, rhs=ckvnew_sb[:, b, :],
                        start=False, stop=True,
                    )
                    nc.vector.tensor_scalar_mul(o_sb[:, b, :], pso, rcp)

                for d in range(N_CORES):
                    nc.sync.dma_start(
                        out=a2a_o_in[d].rearrange("b h c -> h b c"),
                        in_=o_sb[d * HL:(d + 1) * HL],
                    )

            if sim:
                nc.gpsimd.dma_start(a2a_o_out[:], a2a_o_in[:])
            else:
                nc.gpsimd.collective_compute(
                    "AllToAll", mybir.AluOpType.bypass,
                    replica_groups=[list(range(N_CORES))],
                    ins=[a2a_o_in.opt()], outs=[a2a_o_out.opt()],
                )

            # ------------- Phase 3: output projection (head-shard) -----------
            with (
                tc.tile_pool(name="s3", bufs=1) as s3,
                tc.tile_pool(name="ps_t3", bufs=2, space="PSUM") as ps_t3,
                tc.tile_pool(name="ps_r", bufs=2, space="PSUM") as ps_r,
            ):
                ob_sb = s3.tile([B_ALL, HL, C], BF16)
                nc.sync.dma_start(
                    out=ob_sb, in_=a2a_o_out.rearrange("s b h c -> (s b) h c")
                )
                out_sb = s3.tile([B_ALL, HL, V_HEAD], F32)
                oT = s3.tile([128, HL, 4, B_ALL], BF16)
                for h in range(HL):
                    pto = ps_t3.tile([128, 4, B_ALL], BF16, tag="t3")
                    for ct in range(4):
                        nc.tensor.transpose(
                            pto[:, ct, :], ob_sb[:, h, ct * 128:(ct + 1) * 128],
                            ident[:B_ALL, :B_ALL],
                        )
                    nc.vector.tensor_copy(oT[:, h, :, :], pto)
                for h in range(HL):
                    psr = ps_r.tile([B_ALL, V_HEAD], F32, tag="r")
                    for ct in range(4):
                        nc.tensor.matmul(
                            psr, lhsT=oT[:, h, ct, :], rhs=wuv_sb[:, h, ct, :],
                            start=(ct == 0), stop=(ct == 3),
                        )
                    nc.vector.tensor_copy(out_sb[:, h, :], psr)
                nc.sync.dma_start(out=out[:, :, :], in_=out_sb)

    nc.compile()
    return nc


def _get_build(n_cached, *_legacy, sim=False):
    key = (n_cached, sim)
    if key not in _BUILD_CACHE:
        _BUILD_CACHE[key] = _build(n_cached, sim)
    return _BUILD_CACHE[key]


def _rope_half(x, cos, sin):
    """Interleaved-pair rope, numpy: x [..., 64] -> half-split rotated."""
    xe = x[..., 0::2]
    xo = x[..., 1::2]
    return np.concatenate([xe * cos - xo * sin, xo * cos + xe * sin], axis=-1)


def prepare_in_maps(**inputs):
    q = np.asarray(inputs["q_normed_dn"], dtype=np.float32)          # [16,1,1536]
    ckv_new = np.asarray(inputs["compressed_kv"], dtype=np.float32)  # [16,1,512]
    k_pe = np.asarray(inputs["k_pe"], dtype=np.float32)              # [16,1,1,64]
    pos = np.asarray(inputs["position_ids"]).astype(np.int64)        # [16,1]
    start_pos = int(inputs["start_pos"])
    ckv_cache = np.asarray(inputs["ckv_cache"], dtype=np.float32)
    kpe_cache = np.asarray(inputs["k_pe_cache"], dtype=np.float32)
    sin_c = np.asarray(inputs["sin_cache"], dtype=np.float32)
    cos_c = np.asarray(inputs["cos_cache"], dtype=np.float32)
    wkv_b = np.asarray(inputs["wkv_b"], dtype=np.float32)            # [128,256,512]
    wq_b = np.asarray(inputs["wq_b"], dtype=np.float32)              # [24576,1536]

    bsz = q.shape[0]
    n_cached = start_pos
    NT = n_cached // 128
    LT = L // 128

    cos_g = cos_c[pos[:, 0]][:, :32]                                 # [16,32]
    sin_g = sin_c[pos[:, 0]][:, :32]
    cos_rep = np.tile(cos_g * FP8S, (1, HL)).astype(np.float32)      # [16,HL*32]
    sin_rep = np.tile(sin_g * FP8S, (1, HL)).astype(np.float32)

    # q_dn^T arranged to SBUF layout [128, LT, 16]
    qdn_arr = np.ascontiguousarray(
        (q[:, 0, :].T * FP8S).reshape(LT, 128, bsz).transpose(1, 0, 2)
    ).astype(NPF8).reshape(128, LT * bsz)

    wq_r = wq_b.reshape(NUM_HEADS, QD, L)

    # rope the new-token k_pe on host (host-side prep, no device dependency)
    kpe_new_roped = _rope_half(k_pe[:, 0, 0, :], cos_g, sin_g)       # [16, 64]

    in_maps = []
    for core in range(N_CORES):
        hs = slice(core * HL, (core + 1) * HL)
        bs = slice(core * BL, (core + 1) * BL)

        wq_shard = np.ascontiguousarray(
            wq_r[hs].reshape(HL * QD, L).T * FP8S
        ).astype(NPF8)
        wukv_shard = np.ascontiguousarray(wkv_b[hs, :QK_NOPE, :] * FP8S).astype(NPF8)
        # w_uv arranged to SBUF layout [128(c within chunk), HL, 4, 128]
        wuvT = wkv_b[hs, QK_NOPE:, :].transpose(0, 2, 1)             # [HL, C, D]
        wuv_shard = np.ascontiguousarray(
            wuvT.reshape(HL, 4, 128, V_HEAD).transpose(2, 0, 1, 3)
        ).astype(NPBF)

        # fused cache [BL, 640, n]: rows 0-511 ckv^T, 512-575 kpe^T, pad
        fkv_arr = np.zeros((BL, CF, n_cached), dtype=NPF8)
        fkv_arr[:, :C, :] = (
            ckv_cache[bs, :n_cached, :].transpose(0, 2, 1) * FP8S
        ).astype(NPF8)
        fkv_arr[:, C:C + QK_ROPE, :] = (
            kpe_cache[bs, :n_cached, :].transpose(0, 2, 1) * FP8S
        ).astype(NPF8)

        ckv16_shard = np.ascontiguousarray(ckv_cache[bs, :n_cached, :]).astype(NPBF)

        # fnew [128, 5, BL]: scores column for the new token (x16 scale)
        fnew_arr = np.zeros((128, 5, BL), dtype=NPF8)
        fnew_arr[:, 0:4, :] = (
            (ckv_new[bs, 0, :].T * FP8S).reshape(4, 128, BL).transpose(1, 0, 2)
        ).astype(NPF8)
        fnew_arr[:QK_ROPE, 4, :] = (kpe_new_roped[bs].T * FP8S).astype(NPF8)

        ckvnew16 = ckv_new[bs, 0, :].astype(NPBF).reshape(1, BL, C)

        in_maps.append({
            "qdn": qdn_arr,
            "wqT": wq_shard,
            "w_ukv": wukv_shard,
            "w_uvT": wuv_shard,
            "fkv": fkv_arr,
            "ckv16": ckv16_shard,
            "fnew": np.ascontiguousarray(fnew_arr).reshape(128, 5 * BL),
            "ckv_new16": ckvnew16,
            "cosq": cos_rep,
            "sinq": sin_rep,
        })
    return in_maps, (n_cached, bsz)


def assemble(results, meta):
    n_cached, bsz = meta
    out_full = np.empty((bsz, NUM_HEADS, V_HEAD), dtype=np.float32)
    for core in range(N_CORES):
        out_full[:, core * HL:(core + 1) * HL, :] = results[core]["out"]
    return out_full


def kernel(**inputs):
    in_maps, meta = prepare_in_maps(**inputs)
    nc = _get_build(meta[0])
    res = run_bass_kernel_spmd(nc, in_maps, core_ids=list(range(N_CORES)))
    return assemble(res.results, meta)
